# revision 4
# baseline (speedup 1.0000x reference)
"""Trainium2 Bass kernel for nn_DihedralGroupConv.

Math: reference computes
    filt[c,i,d,o] = sum_g perm[g,i,o] * weight[g,c,d]
    out = x.reshape(B,-1) @ filt.reshape(C*2n, D*2n)
i.e. out[b,d,o] = sum_{g,c} weight[g,c,d] * x[b,c, idx_g(o)]
where perm[g] are permutation matrices of the dihedral regular
representation: each is a half-wise cyclic shift of either x itself
(rotations) or of the reflected array xr (reflections).

Kernel strategy (data-parallel over batch, 64 b per core):
  - Host precomputes a halo-padded (216 = 200 + 2*8) per-half image of x,
    laid out directly as the SBUF image AX[128, 16, 2, 216] with
    partition = 32*(b%4) + c, so each generator contribution over a quad
    of 4 batch elements is ONE contiguous-window matmul: rotations read a
    shifted window forward, reflections read a shifted window BACKWARD
    (negative-stride access pattern with swapped halves) -- no second
    image needed.
  - Per quad and generator, four independent 32x32 PE-quadrant matmuls
    (batch element u: SBUF rows 32u -> PSUM partitions 32u) run
    concurrently in the array. All generators accumulate into one PSUM
    bank per quad (8 banks = 8 quads in flight); matmuls are ordered
    gen-major with the two quads of a pair adjacent so each (gen, u)
    weight tile is loaded once per pair and the LDWEIGHTS for tile u
    hides under the other tiles' streams.
  - A short dependency-free warm-up matmul burst (reading a memset
    scratch tile) keeps the PE busy from the instant the framework
    preamble ends, so the HAM clock-gate releases (1.2 -> 2.4 GHz)
    roughly when the real stream starts, without blocking it.
  - DVE/ACT copy PSUM->SBUF staging (cast to fp16), DMA writes a
    scrambled [128, 6400] output image which the host unscrambles.
    Output DMAs alternate between the sync and scalar HWDGE queues;
    the final pair is split into two per-quad DMAs issued in parallel
    on both queues to shorten the kernel tail.
All DMAs are pure 128-partition contiguous-run transfers.
"""

import numpy as np

import concourse.bass as bass  # noqa: F401  (kept for users of this module)
import concourse.mybir as mybir
from concourse import bacc
from concourse.tile import TileContext
from concourse.bass_utils import run_bass_kernel_spmd

# Problem constants (hardcoded per harness contract).
B = 512
C = 32          # in channels
D = 32          # out channels
N = 200         # half length; 2N = 400
L = 2 * N
N_CORES = 8
BPC = B // N_CORES          # 64 batch per core
NQ = BPC // 4               # 16 quads (4 batch / quad)
HALO = 8
PH = N + 2 * HALO           # 216 padded half length

_DT_IN = mybir.dt.float16   # 1 cyc/col PE mode, 1-pass weight load, half DMA
_DT_OUT = mybir.dt.float16  # output staged/stored as fp16, host casts to f32
_NP_IN = np.float16

_cache = {}


def _derive_gens(perm):
    """Classify each generator as (is_refl, shift s) with y[o] = base[(o+s)%N]
    per half, where base is x (rotation) or xr (reflection)."""
    n = N
    o = np.arange(n)
    gens = []
    for g in range(perm.shape[0]):
        idx = np.argmax(perm[g], axis=0).astype(np.int64)  # y[o] = x[idx[o]]
        # rotation candidate: idx[o] = (o - r) % n ; idx[n+o] = n + (o-r)%n
        r = int((-idx[0]) % n)
        rot = np.concatenate([(o - r) % n, n + (o - r) % n])
        if np.array_equal(idx, rot):
            s = -r if r <= n // 2 else n - r
            gens.append((False, s))
            continue
        # reflection candidate: y[o] = xr[(o+r)%n per half] with
        # xr[t] = x[n + (-t)%n], xr[n+t] = x[(-t)%n]
        # => idx[o] = n + (-o-r)%n ; idx[n+o] = (-o-r)%n
        r = int(idx[0] - n) % n     # idx[0] = n + (-r)%n -> (-r)%n
        r = (-r) % n
        refl = np.concatenate([n + (-o - r) % n, (-o - r) % n])
        if np.array_equal(idx, refl):
            s = r if r <= n // 2 else r - n
            gens.append((True, s))
            continue
        raise NotImplementedError(f"perm[{g}] is not a dihedral rep matrix")
    for is_refl, s in gens:
        if is_refl:
            ok = -(HALO - 1) <= s <= HALO
        else:
            ok = -HALO <= s <= HALO
        if not ok:
            raise NotImplementedError(f"shift {s} exceeds halo {HALO}")
    return gens


def _build_program(gens):
    """Build + compile the SPMD Bass program (identical on all cores)."""
    rot = [(j, s) for j, (is_r, s) in enumerate(gens) if not is_r]
    refl = [(j, s) for j, (is_r, s) in enumerate(gens) if is_r]
    nblk = len(rot) + len(refl)

    nc = bacc.Bacc("TRN2", target_bir_lowering=False, debug=False,
                   num_devices=N_CORES, enable_partition_id=False)
    ax_d = nc.dram_tensor("ax", [128, NQ, 2, PH], _DT_IN,
                          kind="ExternalInput")
    ws_d = nc.dram_tensor("ws", [128, 32 * nblk], _DT_IN,
                          kind="ExternalInput")
    outr_d = nc.dram_tensor("outr", [128, NQ * L], _DT_OUT,
                            kind="ExternalOutput")

    CHUNKS = [2, 4, 4, 6]   # DMA chunk sizes along quads (small first
                            # chunk -> matmuls start earlier)
    with TileContext(nc) as tc:
        with (
            tc.tile_pool(name="arrp", bufs=1) as arrp,
            tc.tile_pool(name="wsp", bufs=1) as wsp,
            tc.tile_pool(name="stg", bufs=1) as stgp,
            tc.tile_pool(name="psum", bufs=1, space="PSUM") as psump,
        ):
            ws_sb = wsp.tile([128, 32 * nblk], _DT_IN)
            ax_sb = arrp.tile([128, NQ, 2, PH], _DT_IN, name="ax_sb")
            # weights go on the scalar HWDGE queue so the first data chunk
            # and the weights transfer run concurrently
            nc.scalar.dma_start(out=ws_sb[:, :], in_=ws_d[:, :])
            c0 = 0
            for cq in CHUNKS:
                nc.sync.dma_start(out=ax_sb[:, c0:c0 + cq],
                                  in_=ax_d[:, c0:c0 + cq])
                c0 += cq

            # per-quad PSUM accumulator banks (8 = full PSUM)
            pstiles = [psump.tile([128, L], mybir.dt.float32,
                                  name=f"ps{i}") for i in range(8)]
            stgs = [stgp.tile([128, 2, L], _DT_OUT, name=f"stg{i}")
                    for i in range(4)]

            # HAM warm-up: dependency-free dummy matmuls fill the PE
            # between the end of the framework preamble and the arrival
            # of the first data chunk, so the clock gate is released
            # (2.4 GHz) shortly after the real stream starts.  Sized to
            # ~1.5us at the cold 1.2 GHz so they do not block real work.
            wu_sb = wsp.tile([128, 256], _DT_IN, name="wu_sb")
            nc.gpsimd.memset(wu_sb[:, :], 0.0)
            for _ in range(6):
                nc.tensor.matmul(pstiles[7][:, 0:256], wu_sb[:, 0:128],
                                 wu_sb[:, :], start=True, stop=True)

            # per quad: one matmul per generator per 32x32 PE quadrant
            # (batch element u -> SBUF rows 32u, PSUM partitions 32u);
            # the 4 quadrants run concurrently in the array.
            # (is_refl, weight block col, window param)
            mm_descs = []
            for k, (_, s) in enumerate(rot):
                mm_descs.append((False, 32 * k, s + HALO))
            for k, (_, s) in enumerate(refl):
                mm_descs.append((True, 32 * (len(rot) + k), s))
            ng = len(mm_descs)

            axt = ax_sb[:, :, :, :]
            pstride = axt.ap[0][0]      # free elems per partition

            def rhs_ap(p0, q, is_r, w):
                if not is_r:
                    return ax_sb[p0:p0 + 32, q, :, w:w + N]
                # reflection: swapped halves, backward o scan;
                # out (h, o) reads src[1-h, (HALO+200) - o - s]
                off = p0 * pstride + q * (2 * PH) + PH + (PH - HALO - w)
                return bass.AP(axt.tensor, off,
                               [[pstride, 32], [-PH, 2], [-1, N]])

            # gen-major order with the two quads of a pair adjacent:
            # each (gen, u) weight tile serves two back-to-back matmuls,
            # and tile u's weight load hides under tiles != u streaming.
            for qp in range(NQ // 2):
                qA, qB = 2 * qp, 2 * qp + 1
                psA, psB = pstiles[qA % 8], pstiles[qB % 8]
                for i, (is_r, wc, w) in enumerate(mm_descs):
                    for u in range(4):
                        p0 = 32 * u
                        lhs = ws_sb[p0:p0 + 32, wc:wc + 32]
                        for ps, q in ((psA, qA), (psB, qB)):
                            nc.tensor.matmul(
                                ps[p0:p0 + 32, :], lhs,
                                rhs_ap(p0, q, is_r, w),
                                start=(i == 0), stop=(i == ng - 1),
                                tile_position=(p0, p0),
                                skip_group_check=True,
                            )
                stg = stgs[qp % 4]
                nc.vector.tensor_copy(out=stg[:, 0], in_=psA[:, :])
                nc.scalar.copy(out=stg[:, 1], in_=psB[:, :])
                nc.sync.dma_start(out=outr_d[:, 2 * qp * L:(2 * qp + 2) * L],
                                  in_=stg[:, :, :])
    nc.compile()
    return nc


def _host_images(x, weight, gens):
    """Build per-core AX images and the packed weight image."""
    n = N
    rot = [(j, s) for j, (is_r, s) in enumerate(gens) if not is_r]
    refl = [(j, s) for j, (is_r, s) in enumerate(gens) if is_r]
    nblk = len(rot) + len(refl)

    pad_idx = (np.arange(PH) - HALO) % n
    xh = x.reshape(B, C, 2, n)[:, :, :, pad_idx]          # [B, C, 2, PH]

    ws = np.zeros((128, 32 * nblk), dtype=_NP_IN)
    for k, (j, _) in enumerate(rot + refl):
        for u in range(4):
            ws[32 * u:32 * (u + 1), 32 * k:32 * (k + 1)] = weight[j]

    def img(a, core):
        sl = a[core * BPC:(core + 1) * BPC]               # [64, C, 2, PH]
        out = np.empty((128, NQ, 2, PH), dtype=_NP_IN)
        for u in range(4):
            out[32 * u:32 * (u + 1)] = sl[u::4].transpose(1, 0, 2, 3)
        return np.ascontiguousarray(out)

    axs = [img(xh, c) for c in range(N_CORES)]
    return axs, ws


def _unscramble(outr):
    """outr[32*(b%4)+d, (b>>2)*L + o] -> out shard [BPC, D, L]."""
    r = outr.astype(np.float32).reshape(4, D, NQ, L)    # [b%4, d, q, o]
    r = r.transpose(2, 0, 1, 3)                         # [q, b%4, d, o]
    return np.ascontiguousarray(r.reshape(BPC, D, L))


def kernel(x, weight, perm, _trace=False):
    x = np.asarray(x, dtype=np.float32)
    weight = np.asarray(weight, dtype=np.float32)
    perm = np.asarray(perm, dtype=np.float32)

    gens = _derive_gens(perm)
    key = tuple(gens)
    if key not in _cache:
        _cache[key] = _build_program(gens)
    nc = _cache[key]

    axs, ws = _host_images(x, weight, gens)
    in_maps = [{"ax": axs[c], "ws": ws} for c in range(N_CORES)]
    res = run_bass_kernel_spmd(nc, in_maps, core_ids=list(range(N_CORES)),
                               trace=_trace)
    out = np.concatenate([_unscramble(res.results[c]["outr"])
                          for c in range(N_CORES)], axis=0)
    if _trace:
        kernel.last_exec_time_ns = res.exec_time_ns
        kernel.last_results = res
    return out


# revision 5
# speedup vs baseline: 1.0247x; 1.0247x over previous
"""Trainium2 Bass kernel for nn_DihedralGroupConv.

Math: reference computes
    filt[c,i,d,o] = sum_g perm[g,i,o] * weight[g,c,d]
    out = x.reshape(B,-1) @ filt.reshape(C*2n, D*2n)
i.e. out[b,d,o] = sum_{g,c} weight[g,c,d] * x[b,c, idx_g(o)]
where perm[g] are permutation matrices of the dihedral regular
representation: each is a half-wise cyclic shift of either x itself
(rotations) or of the reflected array xr (reflections).

Kernel strategy (data-parallel over batch, 64 b per core):
  - Host precomputes one flat per-core image [128, 128 + 16*2*216]:
    128 weight columns (4 gens x 32, replicated per batch-slot row
    group) followed by the halo-padded (216 = 200+2*8) per-half image
    of x with partition = 32*(b%4) + c.  Prepending the weights to the
    main stream means they arrive with the first data chunk at full
    DMA bandwidth instead of trickling on a second queue.
  - Per quad and generator, four independent 32x32 PE-quadrant matmuls
    (batch element u: SBUF rows 32u -> PSUM partitions 32u) run
    concurrently in the array; rotations read a shifted window forward,
    reflections read a shifted window BACKWARD (negative-stride AP with
    swapped halves).  All generators accumulate into one PSUM bank per
    quad (8 banks = 8 quads in flight).  Matmuls are ordered gen-major
    with the two quads of a pair adjacent, so each (gen, u) weight tile
    serves two back-to-back matmuls and weight loads hide under the
    other quadrants' streams (measured 42 ns/matmul issue cadence).
  - A dependency-free warm-up matmul burst (reading a memset scratch
    tile) keeps the PE busy from the instant the framework preamble
    ends until the first data chunk lands, so the HAM clock gate
    releases (1.2 -> 2.4 GHz) shortly after the real stream starts.
  - DVE/ACT copy PSUM->SBUF staging (cast to fp16) per pair; one output
    DMA per pair on the sync HWDGE queue.  The final pair is processed
    quad-A-block then quad-B-block so A's drain + DMA overlap B's
    matmuls, shortening the kernel tail.
All DMAs are pure 128-partition contiguous-run transfers; the host
unscrambles the [128, 6400] output image.
"""

import numpy as np

import concourse.bass as bass
import concourse.mybir as mybir
from concourse import bacc
from concourse.tile import TileContext
from concourse.bass_utils import run_bass_kernel_spmd

# Problem constants (hardcoded per harness contract).
B = 512
C = 32          # in channels
D = 32          # out channels
N = 200         # half length; 2N = 400
L = 2 * N
N_CORES = 8
BPC = B // N_CORES          # 64 batch per core
NQ = BPC // 4               # 16 quads (4 batch / quad)
HALO = 8
PH = N + 2 * HALO           # 216 padded half length
QW = 2 * PH                 # flat columns per quad (432)
WS0 = 128                   # weight columns prepended to the image
TOT = WS0 + NQ * QW         # flat image width (7040)

_DT_IN = mybir.dt.float16   # 1 cyc/col PE mode, 1-pass weight load, half DMA
_DT_OUT = mybir.dt.float16  # output staged/stored as fp16, host casts to f32
_NP_IN = np.float16

_cache = {}


def _derive_gens(perm):
    """Classify each generator as (is_refl, shift s) with y[o] = base[(o+s)%N]
    per half, where base is x (rotation) or xr (reflection)."""
    n = N
    o = np.arange(n)
    gens = []
    for g in range(perm.shape[0]):
        idx = np.argmax(perm[g], axis=0).astype(np.int64)  # y[o] = x[idx[o]]
        # rotation candidate: idx[o] = (o - r) % n ; idx[n+o] = n + (o-r)%n
        r = int((-idx[0]) % n)
        rot = np.concatenate([(o - r) % n, n + (o - r) % n])
        if np.array_equal(idx, rot):
            s = -r if r <= n // 2 else n - r
            gens.append((False, s))
            continue
        # reflection candidate: y[o] = xr[(o+r)%n per half] with
        # xr[t] = x[n + (-t)%n], xr[n+t] = x[(-t)%n]
        # => idx[o] = n + (-o-r)%n ; idx[n+o] = (-o-r)%n
        r = int(idx[0] - n) % n     # idx[0] = n + (-r)%n -> (-r)%n
        r = (-r) % n
        refl = np.concatenate([n + (-o - r) % n, (-o - r) % n])
        if np.array_equal(idx, refl):
            s = r if r <= n // 2 else r - n
            gens.append((True, s))
            continue
        raise NotImplementedError(f"perm[{g}] is not a dihedral rep matrix")
    for is_refl, s in gens:
        if is_refl:
            ok = -(HALO - 1) <= s <= HALO
        else:
            ok = -HALO <= s <= HALO
        if not ok:
            raise NotImplementedError(f"shift {s} exceeds halo {HALO}")
    return gens


def _build_program(gens):
    """Build + compile the SPMD Bass program (identical on all cores)."""
    rot = [(j, s) for j, (is_r, s) in enumerate(gens) if not is_r]
    refl = [(j, s) for j, (is_r, s) in enumerate(gens) if is_r]
    nblk = len(rot) + len(refl)
    assert 32 * nblk <= WS0

    nc = bacc.Bacc("TRN2", target_bir_lowering=False, debug=False,
                   num_devices=N_CORES, enable_partition_id=False)
    ax_d = nc.dram_tensor("ax", [128, TOT], _DT_IN, kind="ExternalInput")
    outr_d = nc.dram_tensor("outr", [128, NQ * L], _DT_OUT,
                            kind="ExternalOutput")

    # flat-column DMA chunks: [weights + quads 0-1], then quad groups
    CHUNKS = [WS0 + 2 * QW, 2 * QW, 4 * QW, 4 * QW, 4 * QW]
    assert sum(CHUNKS) == TOT

    with TileContext(nc) as tc:
        with (
            tc.tile_pool(name="arrp", bufs=1) as arrp,
            tc.tile_pool(name="wsp", bufs=1) as wsp,
            tc.tile_pool(name="stg", bufs=1) as stgp,
            tc.tile_pool(name="psum", bufs=1, space="PSUM") as psump,
        ):
            ax_sb = arrp.tile([128, TOT], _DT_IN, name="ax_sb")
            c0 = 0
            for cq in CHUNKS:
                nc.sync.dma_start(out=ax_sb[:, c0:c0 + cq],
                                  in_=ax_d[:, c0:c0 + cq])
                c0 += cq

            # per-quad PSUM accumulator banks (8 = full PSUM)
            pstiles = [psump.tile([128, L], mybir.dt.float32,
                                  name=f"ps{i}") for i in range(8)]
            stgs = [stgp.tile([128, 2, L], _DT_OUT, name=f"stg{i}")
                    for i in range(4)]

            # HAM warm-up: dependency-free dummy matmuls fill the PE from
            # the end of the framework preamble (~7us) until the first
            # data chunk lands (~9.2us), releasing the clock gate shortly
            # after the real stream starts without blocking it.
            wu_sb = wsp.tile([128, 512], _DT_IN, name="wu_sb")
            nc.gpsimd.memset(wu_sb[:, :], 0.0)
            for _ in range(5):
                nc.tensor.matmul(pstiles[7][:, 0:400], wu_sb[:, 0:128],
                                 wu_sb[:, 0:400], start=True, stop=True,
                                 skip_group_check=True)

            # (is_refl, weight block col, window param)
            mm_descs = []
            for k, (_, s) in enumerate(rot):
                mm_descs.append((False, 32 * k, s + HALO))
            for k, (_, s) in enumerate(refl):
                mm_descs.append((True, 32 * (len(rot) + k), s))
            ng = len(mm_descs)

            axt = ax_sb[:, :]
            pstride = axt.ap[0][0]      # free elems per partition (TOT)

            def rhs_ap(p0, q, is_r, w):
                base = p0 * pstride + WS0 + q * QW
                if not is_r:
                    return bass.AP(axt.tensor, base + w,
                                   [[pstride, 32], [PH, 2], [1, N]])
                # reflection: swapped halves, backward o scan;
                # out (h, o) reads src[1-h, (HALO+200) - o - s]
                off = base + PH + (PH - HALO - w)
                return bass.AP(axt.tensor, off,
                               [[pstride, 32], [-PH, 2], [-1, N]])

            def mm(ps, u, q, i):
                is_r, wc, w = mm_descs[i]
                p0 = 32 * u
                nc.tensor.matmul(
                    ps[p0:p0 + 32, :],
                    ax_sb[p0:p0 + 32, wc:wc + 32],
                    rhs_ap(p0, q, is_r, w),
                    start=(i == 0), stop=(i == ng - 1),
                    tile_position=(p0, p0),
                    skip_group_check=True,
                )

            # gen-major order with the two quads of a pair adjacent:
            # each (gen, u) weight tile serves two back-to-back matmuls,
            # and tile u's weight load hides under tiles != u streaming.
            NP = NQ // 2
            for qp in range(NP):
                qA, qB = 2 * qp, 2 * qp + 1
                psA, psB = pstiles[qA % 8], pstiles[qB % 8]
                stg = stgs[qp % 4]
                if qp < NP - 1:
                    for i in range(ng):
                        for u in range(4):
                            mm(psA, u, qA, i)
                            mm(psB, u, qB, i)
                    nc.vector.tensor_copy(out=stg[:, 0], in_=psA[:, :])
                    nc.scalar.copy(out=stg[:, 1], in_=psB[:, :])
                    nc.sync.dma_start(
                        out=outr_d[:, 2 * qp * L:(2 * qp + 2) * L],
                        in_=stg[:, :, :])
                else:
                    # final pair: quad A's full gen block first so its
                    # drain + output DMA overlap quad B's matmuls
                    for i in range(ng):
                        for u in range(4):
                            mm(psA, u, qA, i)
                    nc.vector.tensor_copy(out=stg[:, 0], in_=psA[:, :])
                    nc.sync.dma_start(
                        out=outr_d[:, 2 * qp * L:(2 * qp + 1) * L],
                        in_=stg[:, 0, :])
                    for i in range(ng):
                        for u in range(4):
                            mm(psB, u, qB, i)
                    nc.vector.tensor_copy(out=stg[:, 1], in_=psB[:, :])
                    nc.sync.dma_start(
                        out=outr_d[:, (2 * qp + 1) * L:(2 * qp + 2) * L],
                        in_=stg[:, 1, :])
    nc.compile()
    return nc


def _host_images(x, weight, gens):
    """Build per-core flat images: [ws columns | halo-padded x image]."""
    n = N
    rot = [(j, s) for j, (is_r, s) in enumerate(gens) if not is_r]
    refl = [(j, s) for j, (is_r, s) in enumerate(gens) if is_r]
    nblk = len(rot) + len(refl)

    pad_idx = (np.arange(PH) - HALO) % n
    xh = x.reshape(B, C, 2, n)[:, :, :, pad_idx]          # [B, C, 2, PH]

    ws = np.zeros((128, WS0), dtype=_NP_IN)
    for k, (j, _) in enumerate(rot + refl):
        for u in range(4):
            ws[32 * u:32 * (u + 1), 32 * k:32 * (k + 1)] = weight[j]

    def img(a, core):
        sl = a[core * BPC:(core + 1) * BPC]               # [64, C, 2, PH]
        out = np.empty((128, TOT), dtype=_NP_IN)
        out[:, :WS0] = ws
        for u in range(4):
            out[32 * u:32 * (u + 1), WS0:] = (
                sl[u::4].transpose(1, 0, 2, 3).reshape(32, NQ * QW))
        return np.ascontiguousarray(out)

    axs = [img(xh, c) for c in range(N_CORES)]
    return axs


def _unscramble(outr):
    """outr[32*(b%4)+d, (b>>2)*L + o] -> out shard [BPC, D, L]."""
    r = outr.astype(np.float32).reshape(4, D, NQ, L)    # [b%4, d, q, o]
    r = r.transpose(2, 0, 1, 3)                         # [q, b%4, d, o]
    return np.ascontiguousarray(r.reshape(BPC, D, L))


def kernel(x, weight, perm, _trace=False):
    x = np.asarray(x, dtype=np.float32)
    weight = np.asarray(weight, dtype=np.float32)
    perm = np.asarray(perm, dtype=np.float32)

    gens = _derive_gens(perm)
    key = tuple(gens)
    if key not in _cache:
        _cache[key] = _build_program(gens)
    nc = _cache[key]

    axs = _host_images(x, weight, gens)
    in_maps = [{"ax": axs[c]} for c in range(N_CORES)]
    res = run_bass_kernel_spmd(nc, in_maps, core_ids=list(range(N_CORES)),
                               trace=_trace)
    out = np.concatenate([_unscramble(res.results[c]["outr"])
                          for c in range(N_CORES)], axis=0)
    if _trace:
        kernel.last_exec_time_ns = res.exec_time_ns
        kernel.last_results = res
    return out


# revision 9
# speedup vs baseline: 1.0801x; 1.0540x over previous
"""Trainium2 Bass kernel for nn_DihedralGroupConv.

Math: reference computes
    filt[c,i,d,o] = sum_g perm[g,i,o] * weight[g,c,d]
    out = x.reshape(B,-1) @ filt.reshape(C*2n, D*2n)
i.e. out[b,d,o] = sum_{g,c} weight[g,c,d] * x[b,c, idx_g(o)]
where perm[g] are permutation matrices of the dihedral regular
representation: each is a half-wise cyclic shift of either x itself
(rotations) or of the reflected array xr (reflections).

Kernel strategy (data-parallel over batch, 64 b per core):
  - Host precomputes one flat per-core image [128, 128 + 16*2*216]:
    128 weight columns (4 gens x 32, replicated per batch-slot row
    group) followed by the halo-padded (216 = 200+2*8) per-half image
    of x with partition = 32*(b%4) + c.  Prepending the weights to the
    main stream means they arrive with the first data chunk at full
    DMA bandwidth instead of trickling on a second queue.
  - Per quad and generator, four independent 32x32 PE-quadrant matmuls
    (batch element u: SBUF rows 32u -> PSUM partitions 32u) run
    concurrently in the array; rotations read a shifted window forward,
    reflections read a shifted window BACKWARD (negative-stride AP with
    swapped halves).  All generators accumulate into one PSUM bank per
    quad (8 banks = 8 quads in flight).  Matmuls are ordered gen-major
    with the two quads of a pair adjacent, so each (gen, u) weight tile
    serves two back-to-back matmuls and weight loads hide under the
    other quadrants' streams (measured 42 ns/matmul issue cadence).
  - A dependency-free warm-up matmul burst (reading a memset scratch
    tile) keeps the PE busy from the instant the framework preamble
    ends until the first data chunk lands, so the HAM clock gate
    releases (1.2 -> 2.4 GHz) shortly after the real stream starts.
  - DVE/ACT copy PSUM->SBUF staging (cast to fp16) per pair; one output
    DMA per pair on the sync HWDGE queue.  The final pair is processed
    quad-A-block then quad-B-block so A's drain + DMA overlap B's
    matmuls, shortening the kernel tail.
All DMAs are pure 128-partition contiguous-run transfers; the host
unscrambles the [128, 6400] output image.
"""

import numpy as np

import concourse.bass as bass
import concourse.mybir as mybir
from concourse import bacc
from concourse.tile import TileContext
from concourse.bass_utils import run_bass_kernel_spmd

# Problem constants (hardcoded per harness contract).
B = 512
C = 32          # in channels
D = 32          # out channels
N = 200         # half length; 2N = 400
L = 2 * N
N_CORES = 8
BPC = B // N_CORES          # 64 batch per core
NQ = BPC // 4               # 16 quads (4 batch / quad)
HALO = 8
PH = N + 2 * HALO           # 216 padded half length
QW = 2 * PH                 # flat columns per quad (432)
WS0 = 128                   # weight columns prepended to the image
TOT = WS0 + NQ * QW         # flat image width (7040)

_DT_IN = mybir.dt.float16   # 1 cyc/col PE mode, 1-pass weight load, half DMA
_DT_OUT = mybir.dt.float16  # output staged/stored as fp16, host casts to f32
_NP_IN = np.float16

_cache = {}


def _derive_gens(perm):
    """Classify each generator as (is_refl, shift s) with y[o] = base[(o+s)%N]
    per half, where base is x (rotation) or xr (reflection)."""
    n = N
    o = np.arange(n)
    gens = []
    for g in range(perm.shape[0]):
        idx = np.argmax(perm[g], axis=0).astype(np.int64)  # y[o] = x[idx[o]]
        # rotation candidate: idx[o] = (o - r) % n ; idx[n+o] = n + (o-r)%n
        r = int((-idx[0]) % n)
        rot = np.concatenate([(o - r) % n, n + (o - r) % n])
        if np.array_equal(idx, rot):
            s = -r if r <= n // 2 else n - r
            gens.append((False, s))
            continue
        # reflection candidate: y[o] = xr[(o+r)%n per half] with
        # xr[t] = x[n + (-t)%n], xr[n+t] = x[(-t)%n]
        # => idx[o] = n + (-o-r)%n ; idx[n+o] = (-o-r)%n
        r = int(idx[0] - n) % n     # idx[0] = n + (-r)%n -> (-r)%n
        r = (-r) % n
        refl = np.concatenate([n + (-o - r) % n, (-o - r) % n])
        if np.array_equal(idx, refl):
            s = r if r <= n // 2 else r - n
            gens.append((True, s))
            continue
        raise NotImplementedError(f"perm[{g}] is not a dihedral rep matrix")
    for is_refl, s in gens:
        if is_refl:
            ok = -(HALO - 1) <= s <= HALO
        else:
            ok = -HALO <= s <= HALO
        if not ok:
            raise NotImplementedError(f"shift {s} exceeds halo {HALO}")
    return gens


def _build_program(gens):
    """Build + compile the SPMD Bass program (identical on all cores)."""
    rot = [(j, s) for j, (is_r, s) in enumerate(gens) if not is_r]
    refl = [(j, s) for j, (is_r, s) in enumerate(gens) if is_r]
    nblk = len(rot) + len(refl)
    assert 32 * nblk <= WS0

    nc = bacc.Bacc("TRN2", target_bir_lowering=False, debug=False,
                   num_devices=N_CORES, enable_partition_id=False)
    ax_d = nc.dram_tensor("ax", [128, TOT], _DT_IN, kind="ExternalInput")
    outr_d = nc.dram_tensor("outr", [128, NQ * L], _DT_OUT,
                            kind="ExternalOutput")

    # flat-column DMA chunks: [weights + quad 0], then quad groups.
    # A small first chunk shortens the lead-in: the HWDGE feeds the 16
    # SDMA engines' descriptor rings serially, so the completion sem
    # (inc 16, one per engine) fires ~1.3us after first byte regardless
    # of size -- less data in chunk 1 moves that point earlier.
    CHUNKS = [WS0 + QW, 2 * QW, 4 * QW, 4 * QW, 5 * QW]
    assert sum(CHUNKS) == TOT

    with TileContext(nc) as tc:
        with (
            tc.tile_pool(name="arrp", bufs=1) as arrp,
            tc.tile_pool(name="wsp", bufs=1) as wsp,
            tc.tile_pool(name="stg", bufs=1) as stgp,
            tc.tile_pool(name="psum", bufs=1, space="PSUM") as psump,
        ):
            ax_sb = arrp.tile([128, TOT], _DT_IN, name="ax_sb")
            c0 = 0
            for cq in CHUNKS:
                nc.sync.dma_start(out=ax_sb[:, c0:c0 + cq],
                                  in_=ax_d[:, c0:c0 + cq])
                c0 += cq

            # per-quad PSUM accumulator banks (8 = full PSUM)
            pstiles = [psump.tile([128, L], mybir.dt.float32,
                                  name=f"ps{i}") for i in range(8)]
            stgs = [stgp.tile([128, 2, L], _DT_OUT, name=f"stg{i}")
                    for i in range(4)]

            # HAM warm-up: dependency-free dummy matmuls fill the PE from
            # the end of the framework preamble (~7us) until the first
            # data chunk lands (~9.2us), releasing the clock gate shortly
            # after the real stream starts without blocking it.
            wu_sb = wsp.tile([128, 512], _DT_IN, name="wu_sb")
            nc.gpsimd.memset(wu_sb[:, :], 0.0)
            for _ in range(9):
                nc.tensor.matmul(pstiles[7][:, 0:400], wu_sb[:, 0:128],
                                 wu_sb[:, 0:400], start=True, stop=True,
                                 skip_group_check=True)

            # (is_refl, weight block col, window param)
            mm_descs = []
            for k, (_, s) in enumerate(rot):
                mm_descs.append((False, 32 * k, s + HALO))
            for k, (_, s) in enumerate(refl):
                mm_descs.append((True, 32 * (len(rot) + k), s))
            ng = len(mm_descs)

            axt = ax_sb[:, :]
            pstride = axt.ap[0][0]      # free elems per partition (TOT)

            def rhs_ap(p0, q, is_r, w):
                base = p0 * pstride + WS0 + q * QW
                if not is_r:
                    return bass.AP(axt.tensor, base + w,
                                   [[pstride, 32], [PH, 2], [1, N]])
                # reflection: swapped halves, backward o scan;
                # out (h, o) reads src[1-h, (HALO+200) - o - s]
                off = base + PH + (PH - HALO - w)
                return bass.AP(axt.tensor, off,
                               [[pstride, 32], [-PH, 2], [-1, N]])

            def mm(ps, u, q, i):
                is_r, wc, w = mm_descs[i]
                p0 = 32 * u
                nc.tensor.matmul(
                    ps[p0:p0 + 32, :],
                    ax_sb[p0:p0 + 32, wc:wc + 32],
                    rhs_ap(p0, q, is_r, w),
                    start=(i == 0), stop=(i == ng - 1),
                    tile_position=(p0, p0),
                    skip_group_check=True,
                )

            # Gen-major order with the two quads of a pair adjacent:
            # each (gen, u) weight tile serves two back-to-back matmuls,
            # and tile u's weight load hides under tiles != u streaming.
            # Quad 0 runs solo (only chunk 1 is resident); quad 15 runs
            # solo at the end so the final drain + DMA tail is short.
            groups = [(0,)] + [(q, q + 1) for q in range(1, NQ - 1, 2)] \
                     + [(NQ - 1,)]
            for gi, grp in enumerate(groups):
                stg = stgs[gi % 4]
                for i in range(ng):
                    for u in range(4):
                        for q in grp:
                            mm(pstiles[q % 8], u, q, i)
                if len(grp) == 2:
                    qA, qB = grp
                    nc.vector.tensor_copy(out=stg[:, 0],
                                          in_=pstiles[qA % 8][:, :])
                    nc.scalar.copy(out=stg[:, 1], in_=pstiles[qB % 8][:, :])
                    nc.sync.dma_start(out=outr_d[:, qA * L:(qB + 1) * L],
                                      in_=stg[:, :, :])
                else:
                    q = grp[0]
                    nc.vector.tensor_copy(out=stg[:, 0],
                                          in_=pstiles[q % 8][:, :])
                    nc.sync.dma_start(out=outr_d[:, q * L:(q + 1) * L],
                                      in_=stg[:, 0, :])
    nc.compile()
    return nc


def _host_images(x, weight, gens):
    """Build per-core flat images: [ws columns | halo-padded x image]."""
    n = N
    rot = [(j, s) for j, (is_r, s) in enumerate(gens) if not is_r]
    refl = [(j, s) for j, (is_r, s) in enumerate(gens) if is_r]
    nblk = len(rot) + len(refl)

    pad_idx = (np.arange(PH) - HALO) % n
    xh = x.reshape(B, C, 2, n)[:, :, :, pad_idx]          # [B, C, 2, PH]

    ws = np.zeros((128, WS0), dtype=_NP_IN)
    for k, (j, _) in enumerate(rot + refl):
        for u in range(4):
            ws[32 * u:32 * (u + 1), 32 * k:32 * (k + 1)] = weight[j]

    def img(a, core):
        sl = a[core * BPC:(core + 1) * BPC]               # [64, C, 2, PH]
        out = np.empty((128, TOT), dtype=_NP_IN)
        out[:, :WS0] = ws
        for u in range(4):
            out[32 * u:32 * (u + 1), WS0:] = (
                sl[u::4].transpose(1, 0, 2, 3).reshape(32, NQ * QW))
        return np.ascontiguousarray(out)

    axs = [img(xh, c) for c in range(N_CORES)]
    return axs


def _unscramble(outr):
    """outr[32*(b%4)+d, (b>>2)*L + o] -> out shard [BPC, D, L]."""
    r = outr.astype(np.float32).reshape(4, D, NQ, L)    # [b%4, d, q, o]
    r = r.transpose(2, 0, 1, 3)                         # [q, b%4, d, o]
    return np.ascontiguousarray(r.reshape(BPC, D, L))


def kernel(x, weight, perm, _trace=False):
    x = np.asarray(x, dtype=np.float32)
    weight = np.asarray(weight, dtype=np.float32)
    perm = np.asarray(perm, dtype=np.float32)

    gens = _derive_gens(perm)
    key = tuple(gens)
    if key not in _cache:
        _cache[key] = _build_program(gens)
    nc = _cache[key]

    axs = _host_images(x, weight, gens)
    in_maps = [{"ax": axs[c]} for c in range(N_CORES)]
    res = run_bass_kernel_spmd(nc, in_maps, core_ids=list(range(N_CORES)),
                               trace=_trace)
    out = np.concatenate([_unscramble(res.results[c]["outr"])
                          for c in range(N_CORES)], axis=0)
    if _trace:
        kernel.last_exec_time_ns = res.exec_time_ns
        kernel.last_results = res
    return out
